# revision 5
# baseline (speedup 1.0000x reference)
"""Trainium2 Bass kernel for CausalSelfAttention with external-memory prefix.

Problem shapes (hardcoded): B=2, T=2048, C=1024, H=16, HD=64, MEM=256.
Sharding: 8 cores = 2 (batch) x 4 (head groups of 4 heads).
Host unshards by summing the 4 head-group partials per batch.

Wave-pipelined device algorithm (v2):
  - x processed in 4 t-waves of 512; per wave: PE transposes -> v matmuls
    -> qkT matmuls -> attention for all 4 heads at this t-block, so phase-A
    PE work of wave k+1 overlaps the ACT-bound exp of wave k.
  - No K=1 bias matmuls: host ships pre-broadcast bias tiles (bvb, bpb) and
    the bias add is folded into the existing psum->SBUF copies (DVE/Pool).
  - Diagonal s-tiles trimmed: scores/exp/PV only over non-masked column
    ranges; causal masking is ONE [128,128] bf16 triangle applied with
    strided two-block APs on DVE (2x bf16 mode).
  - P^T = exp(0.125*S^T) written as bf16: PV matmuls cost 1 cycle/row at
    any width (cost keys on the moving operand dtype) and mask mults get
    the DVE 2x mode. Numerator and denominator use the same bf16 probs so
    the quantization error largely cancels after normalization.
  - Softmax denominators on-chip: DVE reciprocal of the psy ones-row ->
    PE ones-broadcast matmul -> DVE scale-mult reading PSUM directly.
"""

import numpy as np
from contextlib import ExitStack

import concourse.bass as bass
import concourse.tile as tile
from concourse import mybir
from concourse import bacc
from concourse import bass_utils

FP32 = mybir.dt.float32
R32 = mybir.dt.float32r
BF16 = mybir.dt.bfloat16
AF = mybir.ActivationFunctionType
ALU = mybir.AluOpType

P = 128
T = 2048
C = 1024
HPC = 4        # heads per core
HD = 64
MEM = 256
S = MEM + T    # 2304
NST = S // P   # 18 s-tiles (0-1 mem, 2-17 causal)


def build_nc() -> bass.Bass:
    nc = bacc.Bacc(
        "TRN2", target_bir_lowering=False, debug=False, num_devices=8
    )
    x_d = nc.dram_tensor("x", (T, C), FP32, kind="ExternalInput").ap()
    # identity fp32 (for PE transposes)
    csto_d = nc.dram_tensor("csto", (P, P), FP32, kind="ExternalInput").ap()
    # causal triangle tri[k, c] = 1 if c >= k else 0, bf16
    tri_d = nc.dram_tensor("tri", (P, P), BF16, kind="ExternalInput").ap()
    wqk_d = nc.dram_tensor("wqk", (C, 512), FP32, kind="ExternalInput").ap()
    wv_d = nc.dram_tensor("wv", (C, 256), FP32, kind="ExternalInput").ap()
    bqk_d = nc.dram_tensor("bqk", (P, 4), FP32, kind="ExternalInput").ap()
    bvb_d = nc.dram_tensor("bvb", (P, 256), FP32, kind="ExternalInput").ap()
    mem_d = nc.dram_tensor("mem", (MEM, 256), FP32, kind="ExternalInput").ap()
    wp_d = nc.dram_tensor("wp", (256, C), FP32, kind="ExternalInput").ap()
    bpb_d = nc.dram_tensor("bpb", (P, C), FP32, kind="ExternalInput").ap()
    out_d = nc.dram_tensor("out", (T, C), FP32, kind="ExternalOutput").ap()

    with tile.TileContext(nc) as tc, ExitStack() as ctx:
        const = ctx.enter_context(tc.tile_pool(name="const", bufs=1))
        big = ctx.enter_context(tc.tile_pool(name="big", bufs=1))
        stage = ctx.enter_context(tc.tile_pool(name="stage", bufs=1))
        xrp = ctx.enter_context(tc.tile_pool(name="xrp", bufs=8))
        xtp = ctx.enter_context(tc.tile_pool(name="xtp", bufs=2))
        qtp = ctx.enter_context(tc.tile_pool(name="qtp", bufs=2))
        ptp = ctx.enter_context(tc.tile_pool(name="ptp", bufs=4))
        osbp = ctx.enter_context(tc.tile_pool(name="osbp", bufs=4))
        rrp = ctx.enter_context(tc.tile_pool(name="rrp", bufs=2))
        btp = ctx.enter_context(tc.tile_pool(name="btp", bufs=2))
        ytp = ctx.enter_context(tc.tile_pool(name="ytp", bufs=2))
        # PSUM: pss pairs 2x2 banks + psy 2 banks + shared pp 2 banks = 8
        ppp = ctx.enter_context(tc.tile_pool(name="ppp", bufs=2, space="PSUM"))
        pssp = ctx.enter_context(tc.tile_pool(name="pss", bufs=2, space="PSUM"))
        psyp = ctx.enter_context(tc.tile_pool(name="psy", bufs=2, space="PSUM"))

        # ---------------- constants / weights ----------------
        csto = const.tile([P, P], FP32)
        tri = const.tile([P, P], BF16)
        bqk_sb = const.tile([P, 4], FP32)
        bvb_sb = const.tile([P, 256], FP32)
        bpb_sb = const.tile([P, C], FP32)
        wqk_sb = const.tile([P, 8, 512], R32)
        wv_sb = const.tile([P, 8, 256], R32)
        wp_sb = const.tile([P, 2, C], R32)
        kTm = const.tile([P, 2, MEM], R32)

        # persistent activations
        # k^T rows per 128-tile: [k h0h1, k h2h3]
        kT = big.tile([P, 2, T], R32)
        # [V | 1] per head, 65 columns each, s on partitions (18 s-tiles)
        vones = big.tile([P, NST, 65 * HPC], BF16)
        yTs = big.tile([P, 2, T], R32)

        # ---- DMA order: x wave0, wv, wqk, mem, small consts, wp, x w1.. ----
        xr_t = [[None] * 8 for _ in range(4)]

        def emit_x_wave_dma(tq):
            for half in range(2):
                for i in range(4):
                    xr = xrp.tile([P, 512], FP32, tag="xr", name="xr")
                    r0 = (tq * 4 + i) * P
                    nc.sync.dma_start(
                        xr, x_d[r0 : r0 + P, half * 512 : (half + 1) * 512]
                    )
                    xr_t[tq][half * 4 + i] = xr

        emit_x_wave_dma(0)
        wv_f = stage.tile([P, 8, 256], FP32, tag="wvf", name="wv_f")
        nc.sync.dma_start(wv_f, wv_d.rearrange("(ko p) n -> p ko n", p=P))
        # wv convert on Pool (idle early; ACT must stay free for wave-0 exp)
        nc.gpsimd.tensor_copy(out=wv_sb, in_=wv_f)
        wqk_f = stage.tile([P, 8, 512], FP32, tag="wqkf", name="wqk_f")
        nc.sync.dma_start(wqk_f, wqk_d.rearrange("(ko p) n -> p ko n", p=P))
        nc.scalar.copy(out=wqk_sb, in_=wqk_f)

        memsb = stage.tile([P, 2, 256], FP32, tag="memsb", name="memsb")
        nc.sync.dma_start(memsb, mem_d.rearrange("(o p) n -> p o n", p=P))
        nc.sync.dma_start(csto, csto_d)
        nc.sync.dma_start(tri, tri_d)
        nc.sync.dma_start(bqk_sb, bqk_d)
        nc.sync.dma_start(bvb_sb, bvb_d)
        nc.sync.dma_start(bpb_sb, bpb_d)
        ident = csto

        wp_f = stage.tile([P, 8, 512], FP32, tag="wpf", name="wp_f")
        for ko in range(2):
            nc.sync.dma_start(
                wp_f[:, ko * 2 : (ko + 1) * 2, :],
                wp_d[ko * P : (ko + 1) * P, :].rearrange(
                    "p (nb n) -> p nb n", n=512
                ),
            )
        nc.gpsimd.tensor_copy(
            out=wp_sb.rearrange("p ko (nb n) -> p ko nb n", n=512),
            in_=wp_f[:, :4, :].rearrange("p (ko nb) n -> p ko nb n", nb=2),
        )
        for tq in range(1, 4):
            emit_x_wave_dma(tq)

        # ones columns of vones (col 64 of each 65-block, all 18 s-tiles)
        nc.vector.memset(
            vones.rearrange("p st (h e) -> p st h e", e=65)[:, :, :, 64:65],
            1.0,
        )
        # mem prefix: V rows into vones s-tiles 0-1, keys transposed into kTm
        for o in range(2):
            nc.vector.tensor_copy(
                out=vones[:, o, :].rearrange("p (h e) -> p h e", e=65)[
                    :, :, :HD
                ],
                in_=memsb[:, o, :].rearrange("p (h e) -> p h e", e=HD),
            )
            for j in range(2):
                pst = ppp.tile([P, 512], FP32, tag="pp", name="pst")
                nc.tensor.transpose(
                    pst[:, :P], memsb[:, o, j * P : (j + 1) * P], ident
                )
                nc.vector.tensor_copy(
                    out=kTm[:, j, o * P : (o + 1) * P], in_=pst[:, :P]
                )

        # ---------------- per-wave emission helpers ----------------
        def emit_transposes(tq, xT):
            # 8 batches of 4 transposes; copies spread over ACT/DVE/Pool
            for ct in range(8):
                ps = ppp.tile([P, 512], FP32, tag="pp", name="pst")
                for i in range(4):
                    nc.tensor.transpose(
                        ps[:, i * P : (i + 1) * P],
                        xr_t[tq][(ct // 4) * 4 + i][
                            :, (ct % 4) * P : (ct % 4 + 1) * P
                        ],
                        ident,
                    )
                dst = xT[:, ct, :]
                if ct % 2 == 0:
                    nc.scalar.copy(out=dst, in_=ps)
                else:
                    nc.vector.tensor_copy(out=dst, in_=ps)

        def emit_v(tq, xT, tt):
            # v = x @ W_v for one t-tile, bias added in the psum->SBUF copy
            psv = ppp.tile([P, 512], FP32, tag="pp", name="psv")
            for ct in range(8):
                nc.tensor.matmul(
                    psv[:, :256],
                    lhsT=xT[:, ct, (tt % 4) * P : (tt % 4 + 1) * P],
                    rhs=wv_sb[:, ct, :],
                    start=(ct == 0),
                    stop=(ct == 7),
                )
            nc.vector.tensor_add(
                out=vones[:, 2 + tt, :].rearrange("p (h e) -> p h e", e=65)[
                    :, :, :HD
                ],
                in0=psv[:, :256].rearrange("p (h e) -> p h e", e=HD),
                in1=bvb_sb.rearrange("p (h e) -> p h e", e=HD),
            )

        def emit_qkT(tq, xT, qT, mt):
            # rows of q^T (mt 0-1) / k^T (mt 2-3) for this wave's 512 cols
            psq = ppp.tile([P, 512], FP32, tag="pp", name="psq")
            for ct in range(8):
                nc.tensor.matmul(
                    psq,
                    lhsT=wqk_sb[:, ct, mt * P : (mt + 1) * P],
                    rhs=xT[:, ct, :],
                    start=(ct == 0),
                    stop=(ct == 7),
                )
            dst = (
                qT[:, mt, :]
                if mt < 2
                else kT[:, mt - 2, tq * 512 : (tq + 1) * 512]
            )
            nc.vector.tensor_scalar_add(dst, psq, bqk_sb[:, mt : mt + 1])

        def kT_slice(h, st):
            base = HD * (h % 2)
            if st < 2:
                return kTm[base : base + HD, h // 2, st * P : (st + 1) * P]
            return kT[base : base + HD, h // 2, (st - 2) * P : (st - 1) * P]

        def emit_attn_head(tq, qT, h):
            base = HD * (h % 2)
            tb = tq
            n_st = 6 + 4 * tb
            n_pair = n_st // 2
            qTh = qT[base : base + HD, h // 2, :]
            psy = psyp.tile([65, 512], FP32, tag="psy", name="psy")
            vo = lambda st: vones[:, st, h * 65 : (h + 1) * 65]
            for pr in range(n_pair):
                st0, st1 = 2 * pr, 2 * pr + 1
                diag0 = st0 - 2 - 4 * tb  # diagonal index of st0, or <0
                pss = pssp.tile([P, 1024], FP32, tag="pss", name="pss")
                pt = ptp.tile([P, 1024], BF16, tag="pt", name="pt")
                if diag0 == 2:
                    # (j2, j3) pair: compute cols [256:512] of each block
                    nc.tensor.matmul(
                        pss[:, 256:512], lhsT=kT_slice(h, st0),
                        rhs=qTh[:, 256:512], start=True, stop=True,
                    )
                    nc.tensor.matmul(
                        pss[:, 768:1024], lhsT=kT_slice(h, st1),
                        rhs=qTh[:, 256:512], start=True, stop=True,
                    )
                    nc.scalar.activation(
                        pt.rearrange("p (a n) -> p a n", a=2)[:, :, 256:512],
                        pss.rearrange("p (a n) -> p a n", a=2)[:, :, 256:512],
                        AF.Exp, scale=0.125,
                    )
                    # triangles at pt cols [256:384] (j2) and [896:1024] (j3)
                    nc.vector.tensor_mul(
                        out=pt[:, 256:384], in0=pt[:, 256:384], in1=tri
                    )
                    nc.vector.tensor_mul(
                        out=pt[:, 896:1024], in0=pt[:, 896:1024], in1=tri
                    )
                    nc.tensor.matmul(
                        psy[:, 256:512], lhsT=vo(st0), rhs=pt[:, 256:512],
                        start=False, stop=False, skip_group_check=True,
                    )
                    nc.tensor.matmul(
                        psy[:, 384:512], lhsT=vo(st1), rhs=pt[:, 896:1024],
                        start=False, stop=True, skip_group_check=True,
                    )
                else:
                    # full pair (mem/causal); j0 trims nothing, j1 to [128:]
                    n1_lo = 128 if diag0 == 0 else 0
                    nc.tensor.matmul(
                        pss[:, :512], lhsT=kT_slice(h, st0), rhs=qTh,
                        start=True, stop=True,
                    )
                    nc.tensor.matmul(
                        pss[:, 512 + n1_lo :], lhsT=kT_slice(h, st1),
                        rhs=qTh[:, n1_lo:], start=True, stop=True,
                    )
                    nc.scalar.activation(pt, pss, AF.Exp, scale=0.125)
                    if diag0 == 0:
                        # triangles at pt cols [0:128] (j0), [640:768] (j1)
                        nc.vector.tensor_mul(
                            out=pt[:, 0:128], in0=pt[:, 0:128], in1=tri
                        )
                        nc.vector.tensor_mul(
                            out=pt[:, 640:768], in0=pt[:, 640:768], in1=tri
                        )
                    nc.tensor.matmul(
                        psy[:, :], lhsT=vo(st0), rhs=pt[:, :512],
                        start=(st0 == 0), stop=False, skip_group_check=True,
                    )
                    nc.tensor.matmul(
                        psy[:, n1_lo:], lhsT=vo(st1), rhs=pt[:, 512 + n1_lo :],
                        start=False, stop=False, skip_group_check=True,
                    )
            # denominator: DVE recip of psy ones-row (p64) -> DMA shift to
            # p0 -> Pool partition_broadcast -> DVE scale-mult from psum
            rr = rrp.tile([P, 512], R32, tag="rr", name="rr")
            with nc.allow_low_precision(reason="fp32r softmax reciprocal"):
                nc.vector.reciprocal(rr[64:65, :], psy[64:65, :])
            nc.sync.dma_start(rr[0:1, :], rr[64:65, :])
            bt = btp.tile([P, 512], R32, tag="bt", name="bt")
            nc.gpsimd.partition_broadcast(bt, rr[0:1, :])
            ydst = yTs[base : base + HD, h // 2, tb * 512 : (tb + 1) * 512]
            if base == 0:
                nc.vector.tensor_mul(out=ydst, in0=psy[:HD, :], in1=bt[:HD, :])
            else:
                yt = ytp.tile([HD, 512], R32, tag="yt", name="yt")
                nc.vector.tensor_mul(out=yt, in0=psy[:HD, :], in1=bt[:HD, :])
                nc.sync.dma_start(ydst, yt)

        def emit_proj_tb(tbp):
            # out = yT^T @ W_proj + b_proj for one t-block's four t-tiles
            for tt in range(4 * tbp, 4 * tbp + 4):
                for nb in range(2):
                    psp = ppp.tile([P, 512], FP32, tag="pp", name="psp")
                    for kt in range(2):
                        nc.tensor.matmul(
                            psp,
                            lhsT=yTs[:, kt, tt * P : (tt + 1) * P],
                            rhs=wp_sb[:, kt, nb * 512 : (nb + 1) * 512],
                            start=(kt == 0),
                            stop=(kt == 1),
                        )
                    osb = osbp.tile([P, 512], FP32, tag="osb", name="osb")
                    eng = nc.vector
                    eng.tensor_add(
                        out=osb, in0=psp,
                        in1=bpb_sb[:, nb * 512 : (nb + 1) * 512],
                    )
                    nc.sync.dma_start(
                        out_d[tt * P : (tt + 1) * P, nb * 512 : (nb + 1) * 512],
                        osb,
                    )

        # ---------------- wave-major schedule ----------------
        for tq in range(4):
            xT = xtp.tile([P, 8, 512], R32, tag="xT", name="xT")
            qT = qtp.tile([P, 2, 512], R32, tag="qT", name="qT")
            emit_transposes(tq, xT)
            for tt in range(4 * tq, 4 * tq + 4):
                emit_v(tq, xT, tt)
            for mt in (0, 2, 1, 3):
                emit_qkT(tq, xT, qT, mt)
            for h in range(HPC):
                emit_attn_head(tq, qT, h)
                if tq >= 1 and h == 1:
                    emit_proj_tb(tq - 1)
        emit_proj_tb(3)

    nc.compile()
    return nc


def _build_csto() -> np.ndarray:
    return np.eye(P, dtype=np.float32)


def _build_tri() -> np.ndarray:
    kk = np.arange(P, dtype=np.int64)[:, None]
    cc = np.arange(P, dtype=np.int64)[None, :]
    return np.where(cc >= kk, 1.0, 0.0).astype(np.float32)


_CSTO = _build_csto()
_TRI = _build_tri()


def shard_inputs(inputs: dict) -> list:
    import ml_dtypes

    x = np.asarray(inputs["x"], dtype=np.float32)
    em = np.asarray(inputs["ext_mem"], dtype=np.float32)
    wa = np.asarray(inputs["W_attn"], dtype=np.float32)
    ba = np.asarray(inputs["b_attn"], dtype=np.float32)
    wp = np.asarray(inputs["W_proj"], dtype=np.float32)
    bp = np.asarray(inputs["b_proj"], dtype=np.float32)
    tri_bf = _TRI.astype(ml_dtypes.bfloat16)

    in_maps = []
    for c in range(8):
        b, g = c // 4, c % 4
        lo = g * 256
        wqk = np.concatenate(
            [wa[:, lo : lo + 256], wa[:, 1024 + lo : 1024 + lo + 256]], axis=1
        )
        bqk = np.concatenate(
            [ba[lo : lo + 256], ba[1024 + lo : 1024 + lo + 256]]
        ).reshape(4, P).T
        bv = ba[2048 + lo : 2048 + lo + 256]
        bpc = bp if g == 0 else np.zeros((C,), np.float32)
        in_maps.append(
            {
                "x": np.ascontiguousarray(x[b]),
                "csto": _CSTO,
                "tri": tri_bf,
                "wqk": np.ascontiguousarray(wqk),
                "wv": np.ascontiguousarray(wa[:, 2048 + lo : 2048 + lo + 256]),
                "bqk": np.ascontiguousarray(bqk),
                "bvb": np.ascontiguousarray(
                    np.broadcast_to(bv[None], (P, 256)).copy()
                ),
                "mem": np.ascontiguousarray(em[b][:, lo : lo + 256]),
                "wp": np.ascontiguousarray(wp[lo : lo + 256, :]),
                "bpb": np.ascontiguousarray(
                    np.broadcast_to(bpc[None], (P, C)).copy()
                ),
            }
        )
    return in_maps


_CACHE: dict = {}


def run_sharded(inputs: dict, trace: bool = False):
    """Returns (full_output [2, T, C], exec_time_ns or None)."""
    nc = _CACHE.get("nc")
    if nc is None:
        nc = build_nc()
        _CACHE["nc"] = nc
    in_maps = shard_inputs(inputs)
    res = bass_utils.run_bass_kernel_spmd(
        nc, in_maps, core_ids=list(range(8)), trace=trace
    )
    parts = [res.results[c]["out"] for c in range(8)]
    full = np.stack(
        [
            parts[0] + parts[1] + parts[2] + parts[3],
            parts[4] + parts[5] + parts[6] + parts[7],
        ]
    ).astype(np.float32)
    return full, res.exec_time_ns


def kernel(**inputs) -> np.ndarray:
    out, _ = run_sharded(inputs, trace=False)
    return out
